# revision 31
# baseline (speedup 1.0000x reference)
"""Compressed (mean-pooled) attention kernel for Trainium2, 8 NeuronCores.

Reference computation (per batch element b):
    K_c = mean-pool(K, 4) ; V_c = mean-pool(V, 4)      # [Sc, D], Sc = S/4
    out = softmax(Q @ K_c^T / sqrt(D)) @ V_c           # [S, D]

Sharding: B=4 batches x 2 query-halves -> 8 cores (data parallel, no
communication).  Each core gets Q[b, h*4096:(h+1)*4096], full K[b], V[b].

Per-core kernel design (all matmuls in bf16, fp32 PSUM accumulate):
  Phase 0: load Q (one DMA per 512-query block), PE-transpose to Q^T [d, q]
    bf16; psum->sbuf copies ride the otherwise-idle ACT engine.
  Phase 1: pool K/V.  Batched contiguous DMAs hold rows 512t+4p+j at
    partition p; the 4-row groups are summed with 2 GpSimd adds + 1 DVE add
    (bf16 out).  Sums, not means: the 1/4 is folded into the exp scale and
    the denominator column.  K_c is PE-transposed into K_cT [d, kc]; V_c
    tiles get an extra column of 4.0 (row-sum trick).
  Phase 2: per 512-query block, for each pair of 128-wide kc chunks:
    scores^T [kc, 1024] = K_cT_chunk^T @ Q^T (PSUM fp32), one ACT exp with
    scale=0.25/sqrt(D) -> bf16 (no max subtraction: |scores| <= ~4, exact
    range for fp32 exp), then 8 accumulating matmuls
    out_j[q, 0:129] += exp_chunk_j^T @ [V_c_chunk | 4.0] into 4 per-subtile
    PSUM banks; column 128 accumulates the softmax denominator.
    Normalize with DVE reciprocal + per-partition scale, one store per block.
Softmax math is exact (matches reference); only matmul operands are bf16.
"""

from contextlib import ExitStack

import numpy as np

import concourse.bass as bass
import concourse.bacc as bacc
import concourse.mybir as mybir
import concourse.tile as tile

F32 = mybir.dt.float32
BF16 = mybir.dt.bfloat16
AX = mybir.AxisListType
AF = mybir.ActivationFunctionType

B, S, D = 4, 8192, 128
R = 4  # compression ratio
N_CORES = 8


def build_nc(s=S, nq=S * B // N_CORES):
    """Build the per-core Bass program (s: K/V rows; nq: queries)."""
    sc = s // R
    n_kc = sc // 128  # 128-wide compressed-key chunks
    qb_size = min(512, nq)
    n_qb = nq // qb_size
    n_sub = qb_size // 128  # 128-query subtiles per block
    group = 2 if n_kc % 2 == 0 else 1  # kc chunks per scores PSUM tile
    n_groups = n_kc // group
    dd = D + 1  # V_c width incl. denominator column

    nc = bacc.Bacc(trn_type="TRN2")
    q_in = nc.declare_dram_parameter("q", [nq, D], F32, isOutput=False)
    k_in = nc.declare_dram_parameter("k", [s, D], F32, isOutput=False)
    v_in = nc.declare_dram_parameter("v", [s, D], F32, isOutput=False)
    ident_in = nc.declare_dram_parameter("ident", [128, 128], F32, isOutput=False)
    out_t = nc.declare_dram_parameter("out", [nq, D], F32, isOutput=True)

    # exp(scale * s): folds the 1/4 pooling mean (K_c holds sums) and the
    # 1/sqrt(D) attention scale.
    scale = float(1.0 / (R * np.sqrt(D)))

    with ExitStack() as ctx:
        tc = ctx.enter_context(tile.TileContext(nc))
        const_p = ctx.enter_context(tc.tile_pool(name="const", bufs=1))
        raw_p = ctx.enter_context(tc.tile_pool(name="raw", bufs=10))
        half_p = ctx.enter_context(tc.tile_pool(name="half", bufs=6))
        kc_p = ctx.enter_context(tc.tile_pool(name="kc", bufs=4))
        big_p = ctx.enter_context(tc.tile_pool(name="big", bufs=1))
        qld_p = ctx.enter_context(tc.tile_pool(name="qld", bufs=8))
        qt_p = ctx.enter_context(tc.tile_pool(name="qt", bufs=8))
        ex_p = ctx.enter_context(tc.tile_pool(name="ex", bufs=44))
        osb_p = ctx.enter_context(tc.tile_pool(name="osb", bufs=4))
        rec_p = ctx.enter_context(tc.tile_pool(name="rec", bufs=8))
        # PSUM: scores tiles [128, group*512] f32 (2 banks) x2 bufs, shared
        # with the [128,128] transpose tiles; 4 per-subtile out accumulators
        # (1 bank each).  4 + 4 = 8 banks.
        ps_s = ctx.enter_context(tc.tile_pool(name="ps_s", bufs=2, space="PSUM"))
        ps_o = ctx.enter_context(tc.tile_pool(name="ps_o", bufs=4, space="PSUM"))

        identf = const_p.tile([128, 128], F32, tag="identf")
        nc.sync.dma_start(identf[:], ident_in[:])
        identb = const_p.tile([128, 128], BF16, tag="identb")
        nc.vector.tensor_copy(identb[:], identf[:])

        zero_bias = const_p.tile([128, 1], F32, tag="zb")
        nc.vector.memset(zero_bias[:], 0.0)
        # Warm the ACT exp table early (one-time ~2.7us table DMA).
        warm = const_p.tile([128, 1], F32, tag="warm")
        nc.scalar.activation(warm[:], zero_bias[:], AF.Exp, bias=zero_bias[:])

        kcT = big_p.tile([128, sc], BF16, tag="kcT")
        vc = big_p.tile([128, n_kc * dd], BF16, tag="vc")

        # ---- Phase 0/1 loads, ordered by when compute needs them:
        # first K chunk -> first-band Q -> rest of K -> rest of Q -> V.
        # The exp ramp only needs K (scores) + the active band's Q^T; V is
        # first consumed by the chains, one band later.
        tpl = min(2, n_kc)
        n_ld = n_kc // tpl
        if n_qb >= 8:
            band_sizes = [2, 2, 2, 1, 1] + [1] * (n_qb - 8)
        else:
            band_sizes = [1] * n_qb
        bands, at = [], 0
        for bs in band_sizes:
            bands.append(list(range(at, at + bs)))
            at += bs

        def load_kv(dram, l, name):
            raw = raw_p.tile([128, tpl * R * D], F32, tag="raw", name=name)
            nc.sync.dma_start(
                raw[:].rearrange("p (t x) -> p t x", t=tpl),
                dram[128 * R * tpl * l : 128 * R * tpl * (l + 1), :].rearrange(
                    "(t p j) d -> p t (j d)", p=128, j=R
                ),
            )
            return raw

        def load_q_dma(qb):
            qld = qld_p.tile([128, n_sub * D], F32, tag="qld", name=f"qld{qb}")
            nc.sync.dma_start(
                qld[:].rearrange("p (i d) -> p i d", d=D),
                q_in[qb * qb_size : (qb + 1) * qb_size, :].rearrange(
                    "(i p) d -> p i d", p=128
                ),
            )
            return qld

        def make_qt(qb):
            qld = qlds[qb]
            qt = qt_p.tile([128, qb_size], BF16, tag="qt", name=f"qt{qb}")
            for i in range(n_sub):
                tpq = ps_s.tile([128, 128], F32, tag="ps_s", name=f"tq{qb}_{i}")
                nc.tensor.transpose(tpq[:], qld[:, 128 * i : 128 * (i + 1)], identf[:])
                nc.vector.tensor_copy(qt[:, 128 * i : 128 * (i + 1)], tpq[:])
            return qt

        kraws, vraws = [], []
        qlds, qts = {}, {}
        kraws.append(load_kv(k_in, 0, "kraw0"))
        for qb in bands[0]:
            qlds[qb] = load_q_dma(qb)
            qts[qb] = make_qt(qb)
        for l in range(1, n_ld):
            kraws.append(load_kv(k_in, l, f"kraw{l}"))
        for qb in range(n_qb):
            if qb not in qlds:
                qlds[qb] = load_q_dma(qb)
        for l in range(n_ld):
            vraws.append(load_kv(v_in, l, f"vraw{l}"))

        def pool4(dst, raw, off, tag):
            """dst[p, 0:D] (bf16) = sum of the 4 j-slices of raw at off."""
            h0 = half_p.tile([128, D], F32, tag="half", name=f"h0{tag}")
            nc.gpsimd.tensor_add(h0[:], raw[:, off : off + D], raw[:, off + D : off + 2 * D])
            h1 = half_p.tile([128, D], F32, tag="half", name=f"h1{tag}")
            nc.gpsimd.tensor_add(
                h1[:], raw[:, off + 2 * D : off + 3 * D], raw[:, off + 3 * D : off + 4 * D]
            )
            with nc.allow_low_precision("4-element pooling sum"):
                nc.vector.tensor_add(dst, h0[:], h1[:])

        def pool_chunk(t):
            """Pool K rows for kc chunk t into kcT."""
            l, ti = divmod(t, tpl)
            off = R * D * ti
            kc_t = kc_p.tile([128, D], BF16, tag="kc")
            pool4(kc_t[:], kraws[l], off, f"k{t}")
            tp = ps_s.tile([128, 128], BF16, tag="ps_s", name=f"tk{t}")
            nc.tensor.transpose(tp[:], kc_t[:], identb[:])
            nc.vector.tensor_copy(kcT[:, 128 * t : 128 * (t + 1)], tp[:])

        def pool_v_chunk(t):
            """Pool V rows for kc chunk t into vc (+ denominator column)."""
            l, ti = divmod(t, tpl)
            off = R * D * ti
            pool4(vc[:, dd * t : dd * t + D], vraws[l], off, f"v{t}")
            nc.gpsimd.memset(vc[:, dd * t + D : dd * (t + 1)], float(R))

        # ---- Phase 2: attention, software-pipelined over bands ----
        # Band-major exp stages keep the pool-trace (slot allocation is
        # strictly in program order) aligned with data arrival; each band's
        # PV chains are interleaved into the NEXT band's exp stage so the PE
        # chain work overlaps ACT exp work.  Chains use the 4 ps_o banks
        # transiently (one query block at a time); exp stages use only ps_s.
        exs = {}

        def chains(qb):
            """PV accumulation + normalize + store for one query block."""
            outp = [
                ps_o.tile([128, dd], F32, tag="ps_o", name=f"o{qb}_{j}")
                for j in range(n_sub)
            ]
            for j in range(n_sub):
                for c in range(n_kc):
                    g, h = divmod(c, group)
                    nc.tensor.matmul(
                        outp[j][:],
                        lhsT=exs[qb, g][
                            :, qb_size * h + 128 * j : qb_size * h + 128 * (j + 1)
                        ],
                        rhs=vc[:, dd * c : dd * (c + 1)],
                        start=(c == 0),
                        stop=(c == n_kc - 1),
                        skip_group_check=True,
                    )
            osb = osb_p.tile([128, n_sub * D], F32, tag="osb")
            for j in range(n_sub):
                rec = rec_p.tile([128, 1], F32, tag="rec")
                nc.vector.reciprocal(rec[:], outp[j][:, D : D + 1])
                nc.vector.tensor_scalar_mul(
                    osb[:, D * j : D * (j + 1)], outp[j][:, 0:D], rec[:]
                )
            nc.sync.dma_start(
                out_t[qb * qb_size : (qb + 1) * qb_size, :].rearrange(
                    "(i p) d -> p i d", p=128
                ),
                osb[:].rearrange("p (i d) -> p i d", d=D),
            )

        pooled = [False] * n_kc
        prev = []
        for bi, band in enumerate(bands):
            last = bi == len(bands) - 1
            for qb in band:
                if qb not in qts:
                    qts[qb] = make_qt(qb)
            # chain i of the previous band fires after group slot(i); for
            # the last band, previous chains run first so the final block
            # can accumulate immediately after each exp (no tail burst).
            slots = {}
            for i in range(len(prev)):
                gslot = 0 if last else i * n_groups // max(len(prev), 1)
                slots.setdefault(gslot, []).append(i)
            outp_last = {}
            if last:
                for qb in band:
                    for j in range(n_sub):
                        outp_last[qb, j] = ps_o.tile(
                            [128, dd], F32, tag="ps_o", name=f"o{qb}_{j}"
                        )
            for g in range(n_groups):
                for i in slots.get(g, []):
                    chains(prev[i])
                for h in range(group):
                    c = group * g + h
                    if not pooled[c]:
                        pool_chunk(c)
                        if len(bands) == 1:
                            pool_v_chunk(c)
                        pooled[c] = True
                for qb in band:
                    sc_ps = ps_s.tile(
                        [128, group * qb_size], F32, tag="ps_s", name=f"s{qb}_{g}"
                    )
                    for h in range(group):
                        c = group * g + h
                        nc.tensor.matmul(
                            sc_ps[:, qb_size * h : qb_size * (h + 1)],
                            lhsT=kcT[:, 128 * c : 128 * (c + 1)],
                            rhs=qts[qb][:],
                            start=True,
                            stop=True,
                        )
                    ex = ex_p.tile(
                        [128, group * qb_size], BF16, tag="ex", name=f"ex{qb}_{g}"
                    )
                    nc.scalar.activation(
                        ex[:], sc_ps[:], AF.Exp, bias=zero_bias[:], scale=scale
                    )
                    exs[qb, g] = ex
                    if last:
                        for h in range(group):
                            c = group * g + h
                            for j in range(n_sub):
                                nc.tensor.matmul(
                                    outp_last[qb, j][:],
                                    lhsT=ex[
                                        :,
                                        qb_size * h + 128 * j : qb_size * h + 128 * (j + 1),
                                    ],
                                    rhs=vc[:, dd * c : dd * (c + 1)],
                                    start=(c == 0),
                                    stop=(c == n_kc - 1),
                                    skip_group_check=True,
                                )
            if bi == 0 and len(bands) > 1:
                # V pooling deferred to here: all vraws have landed by the
                # end of the first exp stage, and the first chain (start of
                # the next band) needs every V chunk.
                for t in range(n_kc):
                    pool_v_chunk(t)
            if last:
                for qb in band:
                    osb = osb_p.tile([128, n_sub * D], F32, tag="osb")
                    for j in range(n_sub):
                        rec = rec_p.tile([128, 1], F32, tag="rec")
                        nc.vector.reciprocal(rec[:], outp_last[qb, j][:, D : D + 1])
                        nc.vector.tensor_scalar_mul(
                            osb[:, D * j : D * (j + 1)], outp_last[qb, j][:, 0:D], rec[:]
                        )
                    nc.sync.dma_start(
                        out_t[qb * qb_size : (qb + 1) * qb_size, :].rearrange(
                            "(i p) d -> p i d", p=128
                        ),
                        osb[:].rearrange("p (i d) -> p i d", d=D),
                    )
            prev = band
    return nc


_NC_CACHE = {}


def _get_nc(s, nq):
    key = (s, nq)
    if key not in _NC_CACHE:
        _NC_CACHE[key] = build_nc(s, nq)
    return _NC_CACHE[key]


def _run(Q, K, V, **spmd_kwargs):
    """Shard across 8 cores, run, gather. Returns (out, BassKernelResults)."""
    from concourse.bass_utils import run_bass_kernel_spmd

    Q = np.ascontiguousarray(np.asarray(Q), dtype=np.float32)
    K = np.ascontiguousarray(np.asarray(K), dtype=np.float32)
    V = np.ascontiguousarray(np.asarray(V), dtype=np.float32)
    b, sl, d = Q.shape
    assert (b, sl, d) == (B, S, D), (b, sl, d)

    half = S // 2  # 4096 queries per core
    ident = np.eye(128, dtype=np.float32)
    in_maps = []
    for c in range(N_CORES):
        bb, h = divmod(c, 2)
        in_maps.append(
            {
                "q": Q[bb, h * half : (h + 1) * half],
                "k": K[bb],
                "v": V[bb],
                "ident": ident,
            }
        )

    nc = _get_nc(S, half)
    if not nc.is_finalized():
        nc.finalize()
    res = run_bass_kernel_spmd(nc, in_maps, core_ids=list(range(N_CORES)), **spmd_kwargs)
    out = np.empty((B, S, D), dtype=np.float32)
    for c in range(N_CORES):
        bb, h = divmod(c, 2)
        out[bb, h * half : (h + 1) * half] = res.results[c]["out"]
    return out, res


def kernel(Q, K, V):
    """Full-input entry point: takes full inputs, returns full output."""
    out, _ = _run(Q, K, V)
    return out


# revision 32
# speedup vs baseline: 1067.1826x; 1067.1826x over previous
"""Compressed (mean-pooled) attention kernel for Trainium2, 8 NeuronCores.

Reference computation (per batch element b):
    K_c = mean-pool(K, 4) ; V_c = mean-pool(V, 4)      # [Sc, D], Sc = S/4
    out = softmax(Q @ K_c^T / sqrt(D)) @ V_c           # [S, D]

Sharding: B=4 batches x 2 query-halves -> 8 cores (data parallel, no
communication).  Each core gets Q[b, h*4096:(h+1)*4096], full K[b], V[b].

Per-core kernel design (all matmuls in bf16, fp32 PSUM accumulate):
  Loads (ordered by consumer): first K chunk -> first-band Q -> rest of K ->
    rest of Q -> V.  K/V DMAs are batched (1024 rows each) with row 512t+4p+j
    at partition p so the compressed index 128t+p stays partition-aligned.
  Pooling: 4-row groups summed with 2 GpSimd adds + 1 DVE add (bf16 out).
    Sums, not means - the 1/4 is folded into the exp scale / denominator
    column.  K_c is PE-transposed into K_cT [d, kc]; V_c gets an extra
    column of 4.0 (denominator trick).  V pooling is deferred until the
    first chains need it.
  Attention, software-pipelined over bands of query blocks (512 queries
  each).  For each band, group by group (pair of 128-wide kc chunks):
    scores^T [kc, 1024] = K_cT_chunk^T @ Q^T (PSUM fp32), one ACT exp op
    with scale=0.25/sqrt(D) -> bf16 (no max subtraction: |scores| <= ~4,
    safely exact for fp32 exp).  The PV chains
    out_j[q, 0:129] += exp_chunk_j^T @ [V_c_chunk | 4.0]
    accumulate per 128-query subtile into transient PSUM banks (column 128
    accumulates the softmax denominator) and are interleaved into the NEXT
    band's exp stage so PE chain work overlaps ACT exp work; the last band
    accumulates immediately after each exp.  Band-major emission keeps
    Tile's strictly in-order pool-slot allocation aligned with data arrival.
    Normalize = DVE reciprocal of column 128 + per-partition scale; one
    batched store per block.
Softmax math matches the reference exactly; only matmul operands are bf16.
"""

from contextlib import ExitStack

import numpy as np

import concourse.bass as bass
import concourse.bacc as bacc
import concourse.mybir as mybir
import concourse.tile as tile

F32 = mybir.dt.float32
BF16 = mybir.dt.bfloat16
AX = mybir.AxisListType
AF = mybir.ActivationFunctionType

B, S, D = 4, 8192, 128
R = 4  # compression ratio
N_CORES = 8


def build_nc(s=S, nq=S * B // N_CORES):
    """Build the per-core Bass program (s: K/V rows; nq: queries)."""
    sc = s // R
    n_kc = sc // 128  # 128-wide compressed-key chunks
    qb_size = min(512, nq)
    n_qb = nq // qb_size
    n_sub = qb_size // 128  # 128-query subtiles per block
    group = 2 if n_kc % 2 == 0 else 1  # kc chunks per scores PSUM tile
    n_groups = n_kc // group
    dd = D + 1  # V_c width incl. denominator column

    nc = bacc.Bacc(trn_type="TRN2")
    q_in = nc.declare_dram_parameter("q", [nq, D], F32, isOutput=False)
    k_in = nc.declare_dram_parameter("k", [s, D], F32, isOutput=False)
    v_in = nc.declare_dram_parameter("v", [s, D], F32, isOutput=False)
    ident_in = nc.declare_dram_parameter("ident", [128, 128], F32, isOutput=False)
    out_t = nc.declare_dram_parameter("out", [nq, D], F32, isOutput=True)

    # exp(scale * s): folds the 1/4 pooling mean (K_c holds sums) and the
    # 1/sqrt(D) attention scale.
    scale = float(1.0 / (R * np.sqrt(D)))

    with ExitStack() as ctx:
        tc = ctx.enter_context(tile.TileContext(nc))
        const_p = ctx.enter_context(tc.tile_pool(name="const", bufs=1))
        raw_p = ctx.enter_context(tc.tile_pool(name="raw", bufs=10))
        half_p = ctx.enter_context(tc.tile_pool(name="half", bufs=6))
        kc_p = ctx.enter_context(tc.tile_pool(name="kc", bufs=4))
        big_p = ctx.enter_context(tc.tile_pool(name="big", bufs=1))
        qld_p = ctx.enter_context(tc.tile_pool(name="qld", bufs=8))
        qt_p = ctx.enter_context(tc.tile_pool(name="qt", bufs=8))
        ex_p = ctx.enter_context(tc.tile_pool(name="ex", bufs=44))
        osb_p = ctx.enter_context(tc.tile_pool(name="osb", bufs=4))
        rec_p = ctx.enter_context(tc.tile_pool(name="rec", bufs=8))
        # PSUM: scores tiles [128, group*512] f32 (2 banks) x2 bufs, shared
        # with the [128,128] transpose tiles; 4 per-subtile out accumulators
        # (1 bank each).  4 + 4 = 8 banks.
        ps_s = ctx.enter_context(tc.tile_pool(name="ps_s", bufs=2, space="PSUM"))
        ps_o = ctx.enter_context(tc.tile_pool(name="ps_o", bufs=4, space="PSUM"))

        identf = const_p.tile([128, 128], F32, tag="identf")
        nc.sync.dma_start(identf[:], ident_in[:])
        identb = const_p.tile([128, 128], BF16, tag="identb")
        nc.vector.tensor_copy(identb[:], identf[:])

        zero_bias = const_p.tile([128, 1], F32, tag="zb")
        nc.vector.memset(zero_bias[:], 0.0)
        # Warm the ACT exp table early (one-time ~2.7us table DMA).
        warm = const_p.tile([128, 1], F32, tag="warm")
        nc.scalar.activation(warm[:], zero_bias[:], AF.Exp, bias=zero_bias[:])

        kcT = big_p.tile([128, sc], BF16, tag="kcT")
        vc = big_p.tile([128, n_kc * dd], BF16, tag="vc")

        # ---- Phase 0/1 loads, ordered by when compute needs them:
        # first K chunk -> first-band Q -> rest of K -> rest of Q -> V.
        # The exp ramp only needs K (scores) + the active band's Q^T; V is
        # first consumed by the chains, one band later.
        tpl = min(2, n_kc)
        n_ld = n_kc // tpl
        if n_qb >= 8:
            band_sizes = [2, 2, 2, 1, 1] + [1] * (n_qb - 8)
        else:
            band_sizes = [1] * n_qb
        bands, at = [], 0
        for bs in band_sizes:
            bands.append(list(range(at, at + bs)))
            at += bs

        def load_kv(dram, l, name):
            raw = raw_p.tile([128, tpl * R * D], F32, tag="raw", name=name)
            nc.sync.dma_start(
                raw[:].rearrange("p (t x) -> p t x", t=tpl),
                dram[128 * R * tpl * l : 128 * R * tpl * (l + 1), :].rearrange(
                    "(t p j) d -> p t (j d)", p=128, j=R
                ),
            )
            return raw

        def load_q_dma(qb):
            qld = qld_p.tile([128, n_sub * D], F32, tag="qld", name=f"qld{qb}")
            nc.sync.dma_start(
                qld[:].rearrange("p (i d) -> p i d", d=D),
                q_in[qb * qb_size : (qb + 1) * qb_size, :].rearrange(
                    "(i p) d -> p i d", p=128
                ),
            )
            return qld

        def make_qt(qb):
            qld = qlds[qb]
            qt = qt_p.tile([128, qb_size], BF16, tag="qt", name=f"qt{qb}")
            for i in range(n_sub):
                tpq = ps_s.tile([128, 128], F32, tag="ps_s", name=f"tq{qb}_{i}")
                nc.tensor.transpose(tpq[:], qld[:, 128 * i : 128 * (i + 1)], identf[:])
                nc.vector.tensor_copy(qt[:, 128 * i : 128 * (i + 1)], tpq[:])
            return qt

        kraws, vraws = [], []
        qlds, qts = {}, {}
        kraws.append(load_kv(k_in, 0, "kraw0"))
        for qb in bands[0]:
            qlds[qb] = load_q_dma(qb)
            qts[qb] = make_qt(qb)
        for l in range(1, n_ld):
            kraws.append(load_kv(k_in, l, f"kraw{l}"))
        for qb in range(n_qb):
            if qb not in qlds:
                qlds[qb] = load_q_dma(qb)
        for l in range(n_ld):
            vraws.append(load_kv(v_in, l, f"vraw{l}"))

        def pool4(dst, raw, off, tag):
            """dst[p, 0:D] (bf16) = sum of the 4 j-slices of raw at off."""
            h0 = half_p.tile([128, D], F32, tag="half", name=f"h0{tag}")
            nc.gpsimd.tensor_add(h0[:], raw[:, off : off + D], raw[:, off + D : off + 2 * D])
            h1 = half_p.tile([128, D], F32, tag="half", name=f"h1{tag}")
            nc.gpsimd.tensor_add(
                h1[:], raw[:, off + 2 * D : off + 3 * D], raw[:, off + 3 * D : off + 4 * D]
            )
            with nc.allow_low_precision("4-element pooling sum"):
                nc.vector.tensor_add(dst, h0[:], h1[:])

        def pool_chunk(t):
            """Pool K rows for kc chunk t into kcT."""
            l, ti = divmod(t, tpl)
            off = R * D * ti
            kc_t = kc_p.tile([128, D], BF16, tag="kc")
            pool4(kc_t[:], kraws[l], off, f"k{t}")
            tp = ps_s.tile([128, 128], BF16, tag="ps_s", name=f"tk{t}")
            nc.tensor.transpose(tp[:], kc_t[:], identb[:])
            nc.vector.tensor_copy(kcT[:, 128 * t : 128 * (t + 1)], tp[:])

        def pool_v_chunk(t):
            """Pool V rows for kc chunk t into vc (+ denominator column)."""
            l, ti = divmod(t, tpl)
            off = R * D * ti
            pool4(vc[:, dd * t : dd * t + D], vraws[l], off, f"v{t}")
            nc.gpsimd.memset(vc[:, dd * t + D : dd * (t + 1)], float(R))

        # ---- Phase 2: attention, software-pipelined over bands ----
        # Band-major exp stages keep the pool-trace (slot allocation is
        # strictly in program order) aligned with data arrival; each band's
        # PV chains are interleaved into the NEXT band's exp stage so the PE
        # chain work overlaps ACT exp work.  Chains use the 4 ps_o banks
        # transiently (one query block at a time); exp stages use only ps_s.
        exs = {}

        def chains(qb):
            """PV accumulation + normalize + store for one query block."""
            outp = [
                ps_o.tile([128, dd], F32, tag="ps_o", name=f"o{qb}_{j}")
                for j in range(n_sub)
            ]
            for j in range(n_sub):
                for c in range(n_kc):
                    g, h = divmod(c, group)
                    nc.tensor.matmul(
                        outp[j][:],
                        lhsT=exs[qb, g][
                            :, qb_size * h + 128 * j : qb_size * h + 128 * (j + 1)
                        ],
                        rhs=vc[:, dd * c : dd * (c + 1)],
                        start=(c == 0),
                        stop=(c == n_kc - 1),
                        skip_group_check=True,
                    )
            osb = osb_p.tile([128, n_sub * D], F32, tag="osb")
            for j in range(n_sub):
                rec = rec_p.tile([128, 1], F32, tag="rec")
                nc.vector.reciprocal(rec[:], outp[j][:, D : D + 1])
                nc.vector.tensor_scalar_mul(
                    osb[:, D * j : D * (j + 1)], outp[j][:, 0:D], rec[:]
                )
            nc.sync.dma_start(
                out_t[qb * qb_size : (qb + 1) * qb_size, :].rearrange(
                    "(i p) d -> p i d", p=128
                ),
                osb[:].rearrange("p (i d) -> p i d", d=D),
            )

        pooled = [False] * n_kc
        prev = []
        for bi, band in enumerate(bands):
            last = bi == len(bands) - 1
            for qb in band:
                if qb not in qts:
                    qts[qb] = make_qt(qb)
            # chain i of the previous band fires after group slot(i); for
            # the last band, previous chains run first so the final block
            # can accumulate immediately after each exp (no tail burst).
            slots = {}
            for i in range(len(prev)):
                gslot = 0 if last else i * n_groups // max(len(prev), 1)
                slots.setdefault(gslot, []).append(i)
            outp_last = {}
            if last:
                for qb in band:
                    for j in range(n_sub):
                        outp_last[qb, j] = ps_o.tile(
                            [128, dd], F32, tag="ps_o", name=f"o{qb}_{j}"
                        )
            for g in range(n_groups):
                for i in slots.get(g, []):
                    chains(prev[i])
                for h in range(group):
                    c = group * g + h
                    if not pooled[c]:
                        pool_chunk(c)
                        if len(bands) == 1:
                            pool_v_chunk(c)
                        pooled[c] = True
                for qb in band:
                    sc_ps = ps_s.tile(
                        [128, group * qb_size], F32, tag="ps_s", name=f"s{qb}_{g}"
                    )
                    for h in range(group):
                        c = group * g + h
                        nc.tensor.matmul(
                            sc_ps[:, qb_size * h : qb_size * (h + 1)],
                            lhsT=kcT[:, 128 * c : 128 * (c + 1)],
                            rhs=qts[qb][:],
                            start=True,
                            stop=True,
                        )
                    ex = ex_p.tile(
                        [128, group * qb_size], BF16, tag="ex", name=f"ex{qb}_{g}"
                    )
                    nc.scalar.activation(
                        ex[:], sc_ps[:], AF.Exp, bias=zero_bias[:], scale=scale
                    )
                    exs[qb, g] = ex
                    if last:
                        for h in range(group):
                            c = group * g + h
                            for j in range(n_sub):
                                nc.tensor.matmul(
                                    outp_last[qb, j][:],
                                    lhsT=ex[
                                        :,
                                        qb_size * h + 128 * j : qb_size * h + 128 * (j + 1),
                                    ],
                                    rhs=vc[:, dd * c : dd * (c + 1)],
                                    start=(c == 0),
                                    stop=(c == n_kc - 1),
                                    skip_group_check=True,
                                )
            if bi == 0 and len(bands) > 1:
                # V pooling deferred to here: all vraws have landed by the
                # end of the first exp stage, and the first chain (start of
                # the next band) needs every V chunk.
                for t in range(n_kc):
                    pool_v_chunk(t)
            if last:
                for qb in band:
                    osb = osb_p.tile([128, n_sub * D], F32, tag="osb")
                    for j in range(n_sub):
                        rec = rec_p.tile([128, 1], F32, tag="rec")
                        nc.vector.reciprocal(rec[:], outp_last[qb, j][:, D : D + 1])
                        nc.vector.tensor_scalar_mul(
                            osb[:, D * j : D * (j + 1)], outp_last[qb, j][:, 0:D], rec[:]
                        )
                    nc.sync.dma_start(
                        out_t[qb * qb_size : (qb + 1) * qb_size, :].rearrange(
                            "(i p) d -> p i d", p=128
                        ),
                        osb[:].rearrange("p (i d) -> p i d", d=D),
                    )
            prev = band
    return nc


_NC_CACHE = {}


def _get_nc(s, nq):
    key = (s, nq)
    if key not in _NC_CACHE:
        _NC_CACHE[key] = build_nc(s, nq)
    return _NC_CACHE[key]


def _run(Q, K, V, **spmd_kwargs):
    """Shard across 8 cores, run, gather. Returns (out, BassKernelResults)."""
    from concourse.bass_utils import run_bass_kernel_spmd

    Q = np.ascontiguousarray(np.asarray(Q), dtype=np.float32)
    K = np.ascontiguousarray(np.asarray(K), dtype=np.float32)
    V = np.ascontiguousarray(np.asarray(V), dtype=np.float32)
    b, sl, d = Q.shape
    assert (b, sl, d) == (B, S, D), (b, sl, d)

    half = S // 2  # 4096 queries per core
    ident = np.eye(128, dtype=np.float32)
    in_maps = []
    for c in range(N_CORES):
        bb, h = divmod(c, 2)
        in_maps.append(
            {
                "q": Q[bb, h * half : (h + 1) * half],
                "k": K[bb],
                "v": V[bb],
                "ident": ident,
            }
        )

    nc = _get_nc(S, half)
    if not nc.is_finalized():
        nc.finalize()
    res = run_bass_kernel_spmd(nc, in_maps, core_ids=list(range(N_CORES)), **spmd_kwargs)
    out = np.empty((B, S, D), dtype=np.float32)
    for c in range(N_CORES):
        bb, h = divmod(c, 2)
        out[bb, h * half : (h + 1) * half] = res.results[c]["out"]
    return out, res


def kernel(Q, K, V):
    """Full-input entry point: takes full inputs, returns full output."""
    out, _ = _run(Q, K, V)
    return out
